# revision 2
# baseline (speedup 1.0000x reference)
"""Trainium2 Bass kernel for a quantized ResNet BasicBlock.

Reference computation (per reference.py):
    out = act_quant(x); out = conv3x3(out, weight_quant(w1)); out = BN(out, g1, b1)
    out = act_quant(out); out = conv3x3(out, weight_quant(w2)); out = BN(out, g2, b2)
    return out + x
with act_quant(x) = round(clip(x,0,1)*15)/15 (4-bit), weight_quant symmetric 4-bit
per-tensor (levels -7..7, scale alpha/7, alpha = max|w|), BN in training mode
(batch stats over (N,H,W)).

Strategy (8 NeuronCores, data-parallel over batch, sync-BN via AllReduce):
  * Quantized activations are integers 0..15, weights integers -7..7 - both
    exact in fp8e4m3, and fp32 PSUM accumulation never rounds, so each conv3x3
    is an EXACT integer computation.
  * Quantization rounds via the hardware's round-to-nearest float->int16
    convert (verified on hw): u1 = round(15*x) is one DVE tensor_scalar
    (f32 -> i16, written straight into the cint tile), u2 =
    round(uscale*y+ubias) one DVE tensor_scalar (i16 -> i16, per-channel
    fp32 scale/bias APs, 4x mode, in place over the conv-1 integers). The
    0..15 clamp+fp8 convert is one more tensor_scalar (i16 -> f8, min/max,
    exact for integers), mostly on GPSIMD.
  * conv3x3 over a zero-padded [C=128, 60, 64] fp8 image: per 8-row output
    group, 6 DoubleRow pair-matmuls contract all 9 taps as K=256 pairs:
    taps (0,dw)+(1,dw) pair naturally (pair stride = one 64B row), taps
    (2,dw) pair with an all-zero weight row (the paired garbage row
    multiplies 0). 42 DR passes per image-conv, no half-rate matmuls, no
    shifted-slab copies.
  * PSUM is split in three tiles (3+2+2 groups): PSUM WAR deps are
    tile-granular, so per-tile drains never gate another tile's matmuls and
    the next image's writes come a full rotation later. Drains run on ACT
    (Identity -> int16, exact) with the per-channel BN sum riding accum_out.
  * BN sum-of-squares is exact and split by rows: 3 DVE scalar_tensor_tensor
    pieces + one ACT Square piece (rows 40-47) that soaks up the ACT slack.
    (Row-subsampled variance was tried and rejected: adjacent-row correlation
    makes per-channel estimates ~3% noisy, an order over the error budget.)
    Per-channel stats are AllReduced across the 8 cores ([128,2] fp32), then
    BN+act_quant collapse into one per-channel scale/bias (Newton-iterated
    rsqrt on DVE, no ACT table reloads).
  * x is loaded as fp32 for quantization (quantizing a bf16 copy flips ~1% of
    the 4-bit levels and blows the error budget); a host-prepared bf16 copy
    of x is DMA-loaded during conv2 (DMA is idle there) as the residual.
  * Finalize: t2 = fscale*y2+fbias, out = t2 + x_bf16, streamed bf16 stores;
    per-image work is spread across DVE/ACT/Pool so the post-BN2 tail tracks
    the store-DMA rate. Dep-free DoubleRow matmuls into a junk PSUM bank
    bridge the BN1 sync bubble so the PE p-state governor never downclocks.
"""

import os
import sys

for _p in ("/opt/trn_rl_repo", "/root/.axon_site/_ro/trn_rl_repo"):
    if os.path.isdir(_p) and _p not in sys.path:
        sys.path.insert(0, _p)

import numpy as np
import ml_dtypes

import concourse.bass as bass  # noqa: F401  (registers types)
import concourse.tile as tile
from concourse import bacc, mybir
from concourse import bass_utils

F32 = mybir.dt.float32
BF16 = mybir.dt.bfloat16
I16 = mybir.dt.int16
F8 = mybir.dt.float8e4
ACTF = mybir.ActivationFunctionType
ALU = mybir.AluOpType
AX = mybir.AxisListType

C = 128
H = W = 56
HP = 60               # padded rows (57 = bottom conv pad, 58-59 anti-NaN for
                      # the zero-pair row overshoot)
WP = 64               # padded cols (16B-aligned rows for fp8 DoubleRow)
GR = 8                # output rows per PSUM group
NG = H // GR          # 7 groups per image
NCORES = 8

# cvec column indices (all [C] fp32, host-computed)
CV_INVM, CV_K1, CV_K2, CV_SM1, CV_SM2, CV_CA1, CV_CB1, CV_CA2, CV_CB2, \
    CV_EPS, CV_NCOLS = range(11)

# phase-1 row chunks (DMA + quant); group g's matmuls need image rows
# <= g*8+8, chunk 0 covering 14 rows releases group 0 immediately
P1CH = [(0, 14), (14, 14), (28, 14), (42, 14)]
# phase-2 row chunks: first chunk 12 rows so group-0 matmuls wait on one
# small chunk only
P2CH = [(0, 12), (12, 16), (28, 16), (44, 12)]


def _bn_coefs(nc, pool, S, SS, cvcol, ph):
    """[C,1] coef math from global integer-unit sum S / sumsq SS to the fused
    scale/bias for this BN + following op.

    ph=1: (uscale, ubias): u = round(conv_int*uscale + ubias) = 15*clip01(BN(y)).
    ph=2: (fscale, fbias): out = conv_int*fscale + fbias = BN(y2).
    """
    idx = [0]

    def mk():
        idx[0] += 1
        return pool.tile([C, 1], F32, tag=f"bc{ph}_{idx[0]}", name=f"bc{ph}_{idx[0]}")

    kcol = CV_K1 if ph == 1 else CV_K2
    smcol = CV_SM1 if ph == 1 else CV_SM2
    acol = CV_CA1 if ph == 1 else CV_CA2
    bcol = CV_CB1 if ph == 1 else CV_CB2

    # critical path: ms -> ms2 -> vpe -> seed -> newton x2 -> scale -> bias;
    # mean / sv / us0 hang off it in parallel (Pool).
    ms = mk()   # mean in real units, = S*(s/m)
    nc.vector.tensor_scalar(ms[:], S, cvcol(smcol), None, op0=ALU.mult)
    sv = mk()   # SS*s^2/m + eps  (on Pool, parallel)
    nc.gpsimd.tensor_scalar(sv[:], SS, cvcol(kcol), cvcol(CV_EPS),
                            op0=ALU.mult, op1=ALU.add)
    mean = mk()  # mean in integer units (only needed for the bias at the end)
    nc.gpsimd.tensor_scalar(mean[:], S, cvcol(CV_INVM), None, op0=ALU.mult)
    ms2 = mk()
    nc.vector.tensor_tensor(out=ms2[:], in0=ms[:], in1=ms[:], op=ALU.mult)
    vpe = mk()  # var_real + eps
    nc.vector.tensor_tensor(out=vpe[:], in0=sv[:], in1=ms2[:], op=ALU.subtract)
    # rsqrt without the scalar engine (avoids act-table reloads):
    # fast-inverse-sqrt seed j = MAGIC - (bits(v) >> 1) computed as
    # MAGIC - 0.5*bits(v) in the fp32 datapath, then two Newton steps
    # r <- r*(1.5 - 0.5*v*r^2) for ~5e-6 rel.
    seed = pool.tile([C, 1], mybir.dt.int32, tag=f"bseed{ph}", name=f"bseed{ph}")
    nc.vector.tensor_scalar(seed[:], vpe[:].bitcast(mybir.dt.int32), -0.5,
                            1597463007.0, op0=ALU.mult, op1=ALU.add)
    r0 = seed[:].bitcast(F32)
    t1 = mk()
    nc.vector.tensor_tensor(out=t1[:], in0=r0, in1=r0, op=ALU.mult)
    nc.vector.tensor_tensor(out=t1[:], in0=t1[:], in1=vpe[:], op=ALU.mult)
    nc.vector.tensor_scalar(t1[:], t1[:], -0.5, 1.5, op0=ALU.mult, op1=ALU.add)
    r1 = mk()
    nc.vector.tensor_tensor(out=r1[:], in0=r0, in1=t1[:], op=ALU.mult)
    # us0 = CA*r1 on Pool, parallel with the second Newton chain
    us0 = mk()
    nc.gpsimd.tensor_tensor(out=us0[:], in0=cvcol(acol), in1=r1[:], op=ALU.mult)
    t2 = mk()
    nc.vector.tensor_tensor(out=t2[:], in0=r1[:], in1=r1[:], op=ALU.mult)
    nc.vector.tensor_tensor(out=t2[:], in0=t2[:], in1=vpe[:], op=ALU.mult)
    nc.vector.tensor_scalar(t2[:], t2[:], -0.5, 1.5, op0=ALU.mult, op1=ALU.add)
    scale = mk()
    nc.vector.tensor_tensor(out=scale[:], in0=us0[:], in1=t2[:], op=ALU.mult)
    mb = mk()
    nc.vector.tensor_tensor(out=mb[:], in0=scale[:], in1=mean[:], op=ALU.mult)
    bias = mk()
    nc.vector.scalar_tensor_tensor(out=bias[:], in0=mb[:], scalar=-1.0,
                                   in1=cvcol(bcol), op0=ALU.mult, op1=ALU.add)
    return scale, bias


def build_program(ncores, nper, collective=True, reps=1):
    nc = bacc.Bacc("TRN2", target_bir_lowering=False, debug=False, num_devices=ncores)

    x_in = nc.dram_tensor("x", [nper, C, H, W], F32, kind="ExternalInput")
    xb_in = nc.dram_tensor("xbf", [nper, C, H, W], BF16, kind="ExternalInput")
    w1_in = nc.dram_tensor("w1s", [C, 12, C], F8, kind="ExternalInput")
    w2_in = nc.dram_tensor("w2s", [C, 12, C], F8, kind="ExternalInput")
    cv_in = nc.dram_tensor("cvec", [C, CV_NCOLS], F32, kind="ExternalInput")
    out_d = nc.dram_tensor("out", [nper, C, H, W], BF16, kind="ExternalOutput")

    with tile.TileContext(nc) as tc:
        with tc.tile_pool(name="const", bufs=1) as cpool, \
             tc.tile_pool(name="apad", bufs=nper) as apool, \
             tc.tile_pool(name="cint", bufs=nper) as ipool, \
             tc.tile_pool(name="xbf", bufs=nper) as xpool, \
             tc.tile_pool(name="xst", bufs=3) as xstg, \
             tc.tile_pool(name="sq", bufs=2) as sqpool, \
             tc.tile_pool(name="fin", bufs=5) as fpool, \
             tc.tile_pool(name="finoff", bufs=3) as fopool, \
             tc.tile_pool(name="stat", bufs=1) as spool, \
             tc.tile_pool(name="psum", bufs=1, space="PSUM") as ppool, \
             tc.tile_pool(name="dram", bufs=1, space="DRAM") as dpool:

            tw1 = cpool.tile([C, 12, C], F8, tag="w1")
            tw2 = cpool.tile([C, 12, C], F8, tag="w2")
            tcv = cpool.tile([C, CV_NCOLS], F32, tag="cv")
            # garbage-operand tile for PE p-state keep-alive matmuls (zeroed;
            # results land in junk PSUM rows and are never read)
            wdum = cpool.tile([C, 2, 512], F8, tag="wdum")
            # cvec via the ACT sequencer so SP's first x chunk issues in
            # parallel at t=0; weight DMAs via Pool's SWDGE
            nc.scalar.dma_start(tcv[:], cv_in.ap())

            def cvcol(j):
                return tcv[:, j:j + 1]

            # dep-free Identity op at t=0 soaks up the one-time act-table load
            warm = cpool.tile([C, 1], F32, tag="warm")
            nc.scalar.activation(warm[:], warm[:], ACTF.Identity, bias=0.0, scale=1.0)
            nc.vector.memset(wdum[:], 0)

            apad = [apool.tile([C, HP, WP], F8, tag="apad", name=f"apad{i}")
                    for i in range(nper)]
            cint = [ipool.tile([C, H, W], I16, tag="cint", name=f"cint{i}")
                    for i in range(nper)]
            xbf = [xpool.tile([C, H, W], BF16, tag="xbf", name=f"xbf{i}")
                   for i in range(nper)]

            rep_ctx = tc.For_i(0, reps, 1) if reps > 1 else None
            if rep_ctx is not None:
                rep_ctx.__enter__()

            # PSUM: three tiles (3+2+2 groups = 7 banks) + 1 junk bank
            GSPLIT = [(0, 3), (3, 2), (5, 2)]    # (first group, ngroups)
            pstiles = [ppool.tile([C, n * GR, WP], F32, tag=f"ps{j}",
                                  name=f"ps{j}", bufs=1)
                       for j, (_, n) in enumerate(GSPLIT)]
            # sumsq row pieces: (row0, rows, on_act)
            DSPLIT = [(0, 24, False), (24, 16, False), (40, 8, True), (48, 8, False)]
            ns_ = 3 * nper       # BN-sum columns (one per drain)
            nss = 4 * nper       # sumsq columns (one per D piece)
            s1p = spool.tile([C, ns_], F32, tag="s1p")
            ss1p = spool.tile([C, nss], F32, tag="ss1p")
            s2p = spool.tile([C, ns_], F32, tag="s1p", name="s2p")
            ss2p = spool.tile([C, nss], F32, tag="ss1p", name="ss2p")

            def pe_keepalive(n):
                out = pstiles[0][:, 0:GR, :].rearrange("c a b -> c (a b)")
                for k in range(n):
                    nc.tensor.matmul(out, wdum[:, :, 0:C], wdum[:],
                                     start=True, stop=True,
                                     perf_mode=mybir.MatmulPerfMode.DoubleRow)

            def emit_group(tw, flat, g, psview):
                """6 DoubleRow pair-matmuls for one 8-row output group:
                p=0..2 pair taps (0,p)+(1,p); p=3..5 pair tap (2,p-3) with an
                all-zero weight row."""
                out = psview.rearrange("c a b -> c (a b)")
                NFLAT = GR * WP
                for p in range(6):
                    dh = 0 if p < 3 else 2
                    dw = p if p < 3 else p - 3
                    base = (g * GR + dh) * WP + dw
                    rhs = flat[:, base:base + NFLAT]
                    rhs.ap.insert(1, [WP, 2])        # [C, 2, 512]
                    nc.tensor.matmul(out, tw[:, 2 * p:2 * p + 2, :], rhs,
                                     start=(p == 0), stop=(p == 5),
                                     perf_mode=mybir.MatmulPerfMode.DoubleRow)

            def conv(i, tw, sp, ssp):
                """conv3x3 of apad[i] into cint[i]; drains on ACT with the BN
                sum riding accum_out; exact sumsq via 3 DVE STT pieces + one
                ACT Square piece."""
                flat = apad[i].rearrange("c h w -> c (h w)")

                for j, (g0, ng) in enumerate(GSPLIT):
                    ps = pstiles[j]
                    for g in range(g0, g0 + ng):
                        emit_group(tw, flat, g, ps[:, (g - g0) * GR:(g - g0 + 1) * GR, :])
                    dst = cint[i][:, g0 * GR:(g0 + ng) * GR, :]
                    nc.scalar.activation(dst, ps[:, 0:ng * GR, 0:W], ACTF.Identity,
                                         bias=0.0, scale=1.0,
                                         accum_out=sp[:, 3 * i + j:3 * i + j + 1])
                for dj, (r0, rows, on_act) in enumerate(DSPLIT):
                    sub = cint[i][:, r0:r0 + rows, :]
                    kk = 4 * i + dj
                    if on_act:
                        sq = sqpool.tile([C, 8, W], BF16, tag="sqa", name="sqa")
                        nc.scalar.activation(sq[:, 0:rows, :], sub, ACTF.Square,
                                             bias=0.0, scale=1.0,
                                             accum_out=ssp[:, kk:kk + 1])
                    else:
                        sq = sqpool.tile([C, 24, W], BF16, tag="sqd", name="sqd")
                        nc.vector.scalar_tensor_tensor(
                            out=sq[:, 0:rows, :], in0=sub, scalar=1.0, in1=sub,
                            op0=ALU.mult, op1=ALU.mult,
                            accum_out=ssp[:, kk:kk + 1])

            def stats_allreduce(sp, ssp, tag):
                st = spool.tile([C, 2], F32, tag=f"st{tag}")
                nc.vector.tensor_reduce(out=st[:, 0:1], in_=sp[:], axis=AX.X, op=ALU.add)
                nc.vector.tensor_reduce(out=st[:, 1:2], in_=ssp[:], axis=AX.X, op=ALU.add)
                if not collective:
                    return st
                din = dpool.tile([C, 2], F32, tag=f"din{tag}")
                dout = dpool.tile([C, 2], F32, tag=f"dout{tag}")
                nc.gpsimd.dma_start(din[:], st[:])
                nc.gpsimd.collective_compute(
                    "AllReduce", ALU.add,
                    replica_groups=[list(range(ncores))],
                    ins=[din.opt()], outs=[dout.opt()])
                gst = spool.tile([C, 2], F32, tag=f"gst{tag}")
                nc.gpsimd.dma_start(gst[:], dout[:])
                return gst

            # ---------------- phase1 (act_quant of x) + conv1 ----------------
            with nc.named_scope("conv1"):

                def phase1_prep(i):
                    # conv borders (written once; the quant stores only touch
                    # the interior): top pad row 0, bottom pad rows 57-59
                    # (57 = conv pad, 58/59 anti-NaN for the zero-pair
                    # overshoot), cols 0 and 57. Cols 58-63 only feed the
                    # unused alignment output columns.
                    nc.vector.memset(apad[i][:, 0, 0:58], 0)
                    nc.vector.memset(apad[i][:, 57:HP, :], 0)
                    nc.vector.memset(apad[i][:, 1:57, 0:1], 0)
                    nc.vector.memset(apad[i][:, 1:57, 57:58], 0)
                    for ci, (r0, rows) in enumerate(P1CH):
                        xt = xstg.tile([C, 14, W], F32, tag="xs", name="xs")
                        xv = xt[:, 0:rows, :]
                        nc.sync.dma_start(xv, x_in.ap()[i][:, r0:r0 + rows, :])
                        # u1 = round(15*x): RNE f32->i16, straight into cint
                        uv = cint[i][:, r0:r0 + rows, :]
                        nc.vector.tensor_scalar(uv, xv, 15.0, None, op0=ALU.mult)
                        # clip to [0,15] + exact int->fp8 convert, on Pool
                        dsta = apad[i][:, 1 + r0:1 + r0 + rows, 1:W + 1]
                        nc.gpsimd.tensor_scalar(dsta, uv, 15, 0, op0=ALU.min, op1=ALU.max)

                pe_keepalive(12)
                phase1_prep(0)
                nc.gpsimd.dma_start(tw1[:], w1_in.ap())
                nc.gpsimd.dma_start(tw2[:], w2_in.ap())
                phase1_prep(1)
                for i in range(nper):
                    conv(i, tw1, s1p, ss1p)
                    if i + 2 < nper:
                        phase1_prep(i + 2)

            # ---------------- BN1 sync + coefs ----------------
            with nc.named_scope("bn1"):
                pe_keepalive(40)
                gst1 = stats_allreduce(s1p, ss1p, 1)
                uscale, ubias = _bn_coefs(nc, spool, gst1[:, 0:1], gst1[:, 1:2], cvcol, 1)

            # ---------------- phase2 (act_quant of BN1) + conv2 ----------------
            with nc.named_scope("conv2"):

                def phase2_prep(i):
                    # residual copy of x arrives as host-prepared bf16 while
                    # the DMA engines are otherwise idle
                    nc.sync.dma_start(xbf[i][:], xb_in.ap()[i])
                    for ci, (r0, rows) in enumerate(P2CH):
                        src = cint[i][:, r0:r0 + rows, :]
                        # u2 = round(uscale*y + ubias), RNE via i16 convert,
                        # in place over the conv1 ints (4x DVE)
                        nc.vector.tensor_scalar(src, src, uscale[:], ubias[:],
                                                op0=ALU.mult, op1=ALU.add)
                        dsta = apad[i][:, 1 + r0:1 + r0 + rows, 1:W + 1]
                        eng = nc.vector if ci == 0 else nc.gpsimd
                        eng.tensor_scalar(dsta, src, 15, 0, op0=ALU.min, op1=ALU.max)

                phase2_prep(0)
                phase2_prep(1)
                phase2_prep(2)
                phase2_prep(3)
                for i in range(nper):
                    conv(i, tw2, s2p, ss2p)
                    if i + 4 < nper:
                        phase2_prep(i + 4)

            # ---------------- BN2 sync + coefs ----------------
            with nc.named_scope("bn2"):
                gst2 = stats_allreduce(s2p, ss2p, 2)
                fscale, fbias = _bn_coefs(nc, spool, gst2[:, 0:1], gst2[:, 1:2], cvcol, 2)

            # ---------------- finalize: BN2 + residual, streamed stores ----------------
            with nc.named_scope("finalize"):
                # per-image engine plan: two offload images run I on ACT and
                # J on Pool (14-row chunks); the rest run both on DVE with
                # 28-row chunks so the tail tracks the store-DMA rate
                PLAN = {2: (True, True), 5: (True, True)}
                for i in range(nper):
                    i_act, j_pool = PLAN.get(i, (False, False))
                    if i_act or j_pool:
                        chunks = [(r, 14) for r in range(0, H, 14)]
                    elif i == 0:
                        chunks = [(0, 14), (14, 14), (28, 28)]
                    else:
                        chunks = [(0, 28), (28, 28)]
                    for r0, rows in chunks:
                        src = cint[i][:, r0:r0 + rows, :]
                        if i_act or j_pool:
                            t2b = fopool.tile([C, 14, W], BF16, tag="fo", name="t2o")
                        else:
                            t2b = fpool.tile([C, 28, W], BF16, tag="fin", name="t2b")
                        t2 = t2b[:, 0:rows, :]
                        if i_act:
                            nc.scalar.activation(t2, src, ACTF.Identity,
                                                 bias=fbias[:], scale=fscale[:])
                        else:
                            nc.vector.tensor_scalar(t2, src, fscale[:], fbias[:],
                                                    op0=ALU.mult, op1=ALU.add)
                        # residual add in place; store completion recycles
                        # the tile
                        aeng = nc.gpsimd if j_pool else nc.vector
                        aeng.tensor_tensor(out=t2, in0=t2,
                                           in1=xbf[i][:, r0:r0 + rows, :], op=ALU.add)
                        nc.sync.dma_start(out_d.ap()[i][:, r0:r0 + rows, :], t2)

            if rep_ctx is not None:
                rep_ctx.__exit__(None, None, None)

    nc.compile()
    return nc


_PROG_CACHE = {}


def _get_program(ncores, nper):
    key = (ncores, nper)
    if key not in _PROG_CACHE:
        _PROG_CACHE[key] = build_program(ncores, nper)
    return _PROG_CACHE[key]


def make_inputs(x, w1, w2, gamma1, beta1, gamma2, beta2, ncores=NCORES):
    """Host-side prep: shard x (fp32 + bf16 residual copy), quantize weights,
    build cvec."""
    x = np.asarray(x, dtype=np.float32)
    n = x.shape[0]
    nper = n // ncores
    assert nper * ncores == n

    def wq(w):
        w = np.asarray(w, dtype=np.float32)
        alpha = np.float32(np.abs(w).max()) + np.float32(1e-12)
        q = np.round(np.clip(w / alpha, -1.0, 1.0) * np.float32(7.0))
        return q.astype(np.float32), np.float32(alpha)

    q1, a1 = wq(w1)
    q2, a2 = wq(w2)
    # [co, ci, kh, kw] -> [ci, 12, co]: 6 DoubleRow pairs
    # [(0,dw),(1,dw)] for dw=0..2 then [(2,dw), zero] for dw=0..2
    f8np = mybir.dt.np(F8)

    def pack(q):
        t = q.transpose(1, 2, 3, 0)     # [ci, kh, kw, co]
        z = np.zeros_like(t[:, 0, 0, :])
        rows = []
        for dw in range(3):
            rows += [t[:, 0, dw, :], t[:, 1, dw, :]]
        for dw in range(3):
            rows += [t[:, 2, dw, :], z]
        return np.ascontiguousarray(np.stack(rows, axis=1)).astype(f8np)

    w1s = pack(q1)
    w2s = pack(q2)
    s1 = np.float32(a1 / np.float32(105.0))
    s2 = np.float32(a2 / np.float32(105.0))
    m = np.float32(n * H * W)

    g1 = np.asarray(gamma1, dtype=np.float32)
    g2 = np.asarray(gamma2, dtype=np.float32)
    b1 = np.asarray(beta1, dtype=np.float32)
    b2 = np.asarray(beta2, dtype=np.float32)

    cvec = np.zeros((C, CV_NCOLS), dtype=np.float32)
    cvec[:, CV_INVM] = np.float32(1.0) / m
    cvec[:, CV_K1] = s1 * s1 / m
    cvec[:, CV_K2] = s2 * s2 / m
    cvec[:, CV_SM1] = s1 / m
    cvec[:, CV_SM2] = s2 / m
    cvec[:, CV_CA1] = np.float32(15.0) * s1 * g1
    cvec[:, CV_CB1] = np.float32(15.0) * b1
    cvec[:, CV_CA2] = s2 * g2
    cvec[:, CV_CB2] = b2
    cvec[:, CV_EPS] = 1e-5

    xbf = x.astype(ml_dtypes.bfloat16)
    in_maps = []
    for c in range(ncores):
        in_maps.append({
            "x": np.ascontiguousarray(x[c * nper:(c + 1) * nper]),
            "xbf": np.ascontiguousarray(xbf[c * nper:(c + 1) * nper]),
            "w1s": w1s, "w2s": w2s, "cvec": cvec,
        })
    return in_maps, nper


def run(x, w1, w2, gamma1, beta1, gamma2, beta2, trace=False):
    in_maps, nper = make_inputs(x, w1, w2, gamma1, beta1, gamma2, beta2)
    nc = _get_program(NCORES, nper)
    res = bass_utils.run_bass_kernel_spmd(
        nc, in_maps, core_ids=list(range(NCORES)), trace=trace)
    out = np.concatenate([r["out"] for r in res.results], axis=0).astype(np.float32)
    return out, res


def kernel(x, w1, w2, gamma1, beta1, gamma2, beta2):
    out, _ = run(x, w1, w2, gamma1, beta1, gamma2, beta2)
    return out


# revision 14
# speedup vs baseline: 1.1796x; 1.1796x over previous
"""Trainium2 Bass kernel for a quantized ResNet BasicBlock.

Reference computation (per reference.py):
    out = act_quant(x); out = conv3x3(out, weight_quant(w1)); out = BN(out, g1, b1)
    out = act_quant(out); out = conv3x3(out, weight_quant(w2)); out = BN(out, g2, b2)
    return out + x
with act_quant(x) = round(clip(x,0,1)*15)/15 (4-bit), weight_quant symmetric 4-bit
per-tensor (levels -7..7, scale alpha/7, alpha = max|w|), BN in training mode
(batch stats over (N,H,W)).

Strategy (8 NeuronCores, data-parallel over batch, sync-BN via AllReduce):
  * Quantized activations are integers 0..15, weights integers -7..7 - both
    exact in fp8e4m3, and fp32 PSUM accumulation never rounds, so each conv3x3
    is an EXACT integer computation.
  * Quantization rounds via the hardware's round-to-nearest float->int16
    convert (verified on hw): u1 = round(15*x) is one DVE tensor_scalar
    (f32 -> i16, written straight into the cint tile), u2 =
    round(uscale*y+ubias) one DVE tensor_scalar (i16 -> i16, per-channel
    fp32 scale/bias APs, 4x mode, in place over the conv-1 integers). The
    0..15 clamp+fp8 convert is one more tensor_scalar (i16 -> f8, min/max,
    exact for integers), mostly on GPSIMD.
  * conv3x3 over a zero-padded [C=128, 60, 64] fp8 image: per 8-row output
    group, 6 DoubleRow pair-matmuls contract all 9 taps as K=256 pairs:
    taps (0,dw)+(1,dw) pair naturally (pair stride = one 64B row), taps
    (2,dw) pair with an all-zero weight row (the paired garbage row
    multiplies 0). 42 DR passes per image-conv, no half-rate matmuls, no
    shifted-slab copies.
  * PSUM is split in three tiles (3+2+2 groups): PSUM WAR deps are
    tile-granular, so per-tile drains never gate another tile's matmuls and
    the next image's writes come a full rotation later. Drains run on ACT
    (Identity -> int16, exact) with the per-channel BN sum riding accum_out.
  * BN sum-of-squares is exact and split by rows: 3 DVE scalar_tensor_tensor
    pieces + one ACT Square piece (rows 40-47) that soaks up the ACT slack.
    (Row-subsampled variance was tried and rejected: adjacent-row correlation
    makes per-channel estimates ~3% noisy, an order over the error budget.)
    Per-channel stats are AllReduced across the 8 cores ([128,2] fp32), then
    BN+act_quant collapse into one per-channel scale/bias (Newton-iterated
    rsqrt on DVE, no ACT table reloads).
  * x is loaded as fp32 for quantization (quantizing a bf16 copy flips ~1% of
    the 4-bit levels and blows the error budget); a host-prepared bf16 copy
    of x is DMA-loaded during conv2 (DMA is idle there) as the residual.
  * Finalize: t2 = fscale*y2+fbias, out = t2 + x_bf16, streamed bf16 stores;
    per-image work is spread across DVE/ACT/Pool so the post-BN2 tail tracks
    the store-DMA rate. Dep-free DoubleRow matmuls into a junk PSUM bank
    bridge the BN1 sync bubble so the PE p-state governor never downclocks.
"""

import os
import sys

for _p in ("/opt/trn_rl_repo", "/root/.axon_site/_ro/trn_rl_repo"):
    if os.path.isdir(_p) and _p not in sys.path:
        sys.path.insert(0, _p)

import numpy as np
import ml_dtypes

import concourse.bass as bass  # noqa: F401  (registers types)
import concourse.tile as tile
from concourse import bacc, mybir
from concourse import bass_utils

F32 = mybir.dt.float32
BF16 = mybir.dt.bfloat16
I16 = mybir.dt.int16
F8 = mybir.dt.float8e4
ACTF = mybir.ActivationFunctionType
ALU = mybir.AluOpType
AX = mybir.AxisListType

C = 128
H = W = 56
HP = 60               # padded rows (57 = bottom conv pad, 58-59 anti-NaN for
                      # the zero-pair row overshoot)
WP = 64               # padded cols (16B-aligned rows for fp8 DoubleRow)
GR = 8                # output rows per PSUM group
NG = H // GR          # 7 groups per image
NCORES = 8

# cvec column indices (all [C] fp32, host-computed)
CV_INVM, CV_K1, CV_K2, CV_SM1, CV_SM2, CV_CA1, CV_CB1, CV_CA2, CV_CB2, \
    CV_EPS, CV_NCOLS = range(11)

# phase-1 row chunks (DMA + quant); group g's matmuls need image rows
# <= g*8+8, chunk 0 covering 14 rows releases group 0 immediately
P1CH = [(0, 14), (14, 14), (28, 14), (42, 14)]
# phase-2 row chunks: first chunk 12 rows so group-0 matmuls wait on one
# small chunk only
P2CH = [(0, 12), (12, 16), (28, 16), (44, 12)]


def _bn_coefs(nc, pool, S, SS, cvcol, ph):
    """[C,1] coef math from global integer-unit sum S / sumsq SS to the fused
    scale/bias for this BN + following op.

    ph=1: (uscale, ubias): u = round(conv_int*uscale + ubias) = 15*clip01(BN(y)).
    ph=2: (fscale, fbias): out = conv_int*fscale + fbias = BN(y2).
    """
    idx = [0]

    def mk():
        idx[0] += 1
        return pool.tile([C, 1], F32, tag=f"bc{ph}_{idx[0]}", name=f"bc{ph}_{idx[0]}")

    kcol = CV_K1 if ph == 1 else CV_K2
    smcol = CV_SM1 if ph == 1 else CV_SM2
    acol = CV_CA1 if ph == 1 else CV_CA2
    bcol = CV_CB1 if ph == 1 else CV_CB2

    # critical path: ms -> ms2 -> vpe -> seed -> newton x2 -> scale -> bias;
    # mean / sv / us0 hang off it in parallel (Pool).
    ms = mk()   # mean in real units, = S*(s/m)
    nc.vector.tensor_scalar(ms[:], S, cvcol(smcol), None, op0=ALU.mult)
    sv = mk()   # SS*s^2/m + eps  (on Pool, parallel)
    nc.gpsimd.tensor_scalar(sv[:], SS, cvcol(kcol), cvcol(CV_EPS),
                            op0=ALU.mult, op1=ALU.add)
    mean = mk()  # mean in integer units (only needed for the bias at the end)
    nc.gpsimd.tensor_scalar(mean[:], S, cvcol(CV_INVM), None, op0=ALU.mult)
    ms2 = mk()
    nc.vector.tensor_tensor(out=ms2[:], in0=ms[:], in1=ms[:], op=ALU.mult)
    vpe = mk()  # var_real + eps
    nc.vector.tensor_tensor(out=vpe[:], in0=sv[:], in1=ms2[:], op=ALU.subtract)
    # rsqrt without the scalar engine (avoids act-table reloads):
    # fast-inverse-sqrt seed j = MAGIC - (bits(v) >> 1) computed as
    # MAGIC - 0.5*bits(v) in the fp32 datapath, then two Newton steps
    # r <- r*(1.5 - 0.5*v*r^2) for ~5e-6 rel.
    seed = pool.tile([C, 1], mybir.dt.int32, tag=f"bseed{ph}", name=f"bseed{ph}")
    nc.vector.tensor_scalar(seed[:], vpe[:].bitcast(mybir.dt.int32), -0.5,
                            1597463007.0, op0=ALU.mult, op1=ALU.add)
    r0 = seed[:].bitcast(F32)
    t1 = mk()
    nc.vector.tensor_tensor(out=t1[:], in0=r0, in1=r0, op=ALU.mult)
    nc.vector.tensor_tensor(out=t1[:], in0=t1[:], in1=vpe[:], op=ALU.mult)
    nc.vector.tensor_scalar(t1[:], t1[:], -0.5, 1.5, op0=ALU.mult, op1=ALU.add)
    r1 = mk()
    nc.vector.tensor_tensor(out=r1[:], in0=r0, in1=t1[:], op=ALU.mult)
    # us0 = CA*r1 on Pool, parallel with the second Newton chain
    us0 = mk()
    nc.gpsimd.tensor_tensor(out=us0[:], in0=cvcol(acol), in1=r1[:], op=ALU.mult)
    t2 = mk()
    nc.vector.tensor_tensor(out=t2[:], in0=r1[:], in1=r1[:], op=ALU.mult)
    nc.vector.tensor_tensor(out=t2[:], in0=t2[:], in1=vpe[:], op=ALU.mult)
    nc.vector.tensor_scalar(t2[:], t2[:], -0.5, 1.5, op0=ALU.mult, op1=ALU.add)
    scale = mk()
    nc.vector.tensor_tensor(out=scale[:], in0=us0[:], in1=t2[:], op=ALU.mult)
    mb = mk()
    nc.vector.tensor_tensor(out=mb[:], in0=scale[:], in1=mean[:], op=ALU.mult)
    bias = mk()
    nc.vector.scalar_tensor_tensor(out=bias[:], in0=mb[:], scalar=-1.0,
                                   in1=cvcol(bcol), op0=ALU.mult, op1=ALU.add)
    return scale, bias


def build_program(ncores, nper, collective=True, reps=1):
    nc = bacc.Bacc("TRN2", target_bir_lowering=False, debug=False, num_devices=ncores)

    x_in = nc.dram_tensor("x", [nper, C, H, W], F32, kind="ExternalInput")
    xb_in = nc.dram_tensor("xbf", [nper, C, H, W], BF16, kind="ExternalInput")
    w1_in = nc.dram_tensor("w1s", [C, 12, C], F8, kind="ExternalInput")
    w2_in = nc.dram_tensor("w2s", [C, 12, C], F8, kind="ExternalInput")
    cv_in = nc.dram_tensor("cvec", [C, CV_NCOLS], F32, kind="ExternalInput")
    out_d = nc.dram_tensor("out", [nper, C, H, W], BF16, kind="ExternalOutput")

    with tile.TileContext(nc) as tc:
        with tc.tile_pool(name="const", bufs=1) as cpool, \
             tc.tile_pool(name="apad", bufs=nper) as apool, \
             tc.tile_pool(name="cint", bufs=nper) as ipool, \
             tc.tile_pool(name="xbf", bufs=nper) as xpool, \
             tc.tile_pool(name="xst", bufs=5) as xstg, \
             tc.tile_pool(name="sq", bufs=2) as sqpool, \
             tc.tile_pool(name="fin", bufs=4) as fpool, \
             tc.tile_pool(name="finoff", bufs=3) as fopool, \
             tc.tile_pool(name="stat", bufs=1) as spool, \
             tc.tile_pool(name="psum", bufs=1, space="PSUM") as ppool, \
             tc.tile_pool(name="dram", bufs=1, space="DRAM") as dpool:

            tw1 = cpool.tile([C, 12, C], F8, tag="w1")
            tw2 = cpool.tile([C, 12, C], F8, tag="w2")
            tcv = cpool.tile([C, CV_NCOLS], F32, tag="cv")
            # garbage-operand tile for PE p-state keep-alive matmuls (zeroed;
            # results land in junk PSUM rows and are never read)
            wdum = cpool.tile([C, 2, 512], F8, tag="wdum")
            # cvec via the ACT sequencer so SP's first x chunk issues in
            # parallel at t=0; weight DMAs via Pool's SWDGE
            nc.scalar.dma_start(tcv[:], cv_in.ap())

            def cvcol(j):
                return tcv[:, j:j + 1]

            # dep-free Identity op at t=0 soaks up the one-time act-table load
            warm = cpool.tile([C, 1], F32, tag="warm")
            nc.scalar.activation(warm[:], warm[:], ACTF.Identity, bias=0.0, scale=1.0)
            nc.vector.memset(wdum[:], 0)

            apad = [apool.tile([C, HP, WP], F8, tag="apad", name=f"apad{i}")
                    for i in range(nper)]
            cint = [ipool.tile([C, H, W], I16, tag="cint", name=f"cint{i}")
                    for i in range(nper)]
            xbf = [xpool.tile([C, H, W], BF16, tag="xbf", name=f"xbf{i}")
                   for i in range(nper)]

            rep_ctx = tc.For_i(0, reps, 1) if reps > 1 else None
            if rep_ctx is not None:
                rep_ctx.__enter__()

            # PSUM: three tiles (3+2+2 groups = 7 banks) + 1 junk bank
            GSPLIT = [(0, 3), (3, 2), (5, 2)]    # (first group, ngroups)
            pstiles = [ppool.tile([C, n * GR, WP], F32, tag=f"ps{j}",
                                  name=f"ps{j}", bufs=1)
                       for j, (_, n) in enumerate(GSPLIT)]
            # sumsq row pieces: (row0, rows, engine) per phase; phase 1's
            # last piece rides Pool (its B-chunks leave slack), phase 2's
            # stays on DVE (Pool carries more F there)
            DSPLIT1 = [(0, 24, 'v'), (24, 16, 'v'), (40, 8, 'a'), (48, 8, 'v')]
            DSPLIT2 = [(0, 24, 'v'), (24, 16, 'v'), (40, 8, 'a'), (48, 8, 'v')]
            ns_ = 3 * nper       # BN-sum columns (one per drain)
            nss = 4 * nper       # sumsq columns (one per D piece)
            s1p = spool.tile([C, ns_], F32, tag="s1p")
            ss1p = spool.tile([C, nss], F32, tag="ss1p")
            s2p = spool.tile([C, ns_], F32, tag="s1p", name="s2p")
            ss2p = spool.tile([C, nss], F32, tag="ss1p", name="ss2p")

            def pe_keepalive(n):
                out = pstiles[0][:, 0:GR, :].rearrange("c a b -> c (a b)")
                for k in range(n):
                    nc.tensor.matmul(out, wdum[:, :, 0:C], wdum[:],
                                     start=True, stop=True,
                                     perf_mode=mybir.MatmulPerfMode.DoubleRow)

            def emit_group(tw, flat, g, psview):
                """6 DoubleRow pair-matmuls for one 8-row output group:
                p=0..2 pair taps (0,p)+(1,p); p=3..5 pair tap (2,p-3) with an
                all-zero weight row."""
                out = psview.rearrange("c a b -> c (a b)")
                NFLAT = GR * WP
                for p in range(6):
                    dh = 0 if p < 3 else 2
                    dw = p if p < 3 else p - 3
                    base = (g * GR + dh) * WP + dw
                    rhs = flat[:, base:base + NFLAT]
                    rhs.ap.insert(1, [WP, 2])        # [C, 2, 512]
                    nc.tensor.matmul(out, tw[:, 2 * p:2 * p + 2, :], rhs,
                                     start=(p == 0), stop=(p == 5),
                                     perf_mode=mybir.MatmulPerfMode.DoubleRow)

            def conv(i, tw, sp):
                """conv3x3 of apad[i] into cint[i]; drains on ACT with the BN
                sum riding accum_out."""
                flat = apad[i].rearrange("c h w -> c (h w)")
                for j, (g0, ng) in enumerate(GSPLIT):
                    ps = pstiles[j]
                    for g in range(g0, g0 + ng):
                        emit_group(tw, flat, g, ps[:, (g - g0) * GR:(g - g0 + 1) * GR, :])
                    dst = cint[i][:, g0 * GR:(g0 + ng) * GR, :]
                    nc.scalar.activation(dst, ps[:, 0:ng * GR, 0:W], ACTF.Identity,
                                         bias=0.0, scale=1.0,
                                         accum_out=sp[:, 3 * i + j:3 * i + j + 1])

            def sumsq(i, ssp, dsplit):
                """Exact sum-of-squares pieces over cint[i]; emitted a couple
                of images late so the drain-gated pieces never head-of-line
                block the next images' quant ops in the DVE/Pool queues."""
                for dj, (r0, rows, eng) in enumerate(dsplit):
                    sub = cint[i][:, r0:r0 + rows, :]
                    kk = 4 * i + dj
                    if eng == 'a':
                        sq = sqpool.tile([C, 8, W], BF16, tag="sqa", name="sqa")
                        nc.scalar.activation(sq[:, 0:rows, :], sub, ACTF.Square,
                                             bias=0.0, scale=1.0,
                                             accum_out=ssp[:, kk:kk + 1])
                    else:
                        sq = sqpool.tile([C, 24, W], BF16, tag="sqd", name="sqd")
                        e = nc.vector if eng == 'v' else nc.gpsimd
                        e.scalar_tensor_tensor(
                            out=sq[:, 0:rows, :], in0=sub, scalar=1.0, in1=sub,
                            op0=ALU.mult, op1=ALU.mult,
                            accum_out=ssp[:, kk:kk + 1])

            def stats_allreduce(sp, ssp, tag):
                st = spool.tile([C, 2], F32, tag=f"st{tag}")
                nc.vector.tensor_reduce(out=st[:, 0:1], in_=sp[:], axis=AX.X, op=ALU.add)
                nc.vector.tensor_reduce(out=st[:, 1:2], in_=ssp[:], axis=AX.X, op=ALU.add)
                if not collective:
                    return st
                din = dpool.tile([C, 2], F32, tag=f"din{tag}")
                dout = dpool.tile([C, 2], F32, tag=f"dout{tag}")
                nc.gpsimd.dma_start(din[:], st[:])
                nc.gpsimd.collective_compute(
                    "AllReduce", ALU.add,
                    replica_groups=[list(range(ncores))],
                    ins=[din.opt()], outs=[dout.opt()])
                gst = spool.tile([C, 2], F32, tag=f"gst{tag}")
                nc.gpsimd.dma_start(gst[:], dout[:])
                return gst

            # ---------------- phase1 (act_quant of x) + conv1 ----------------
            with nc.named_scope("conv1"):

                def phase1_prep(i):
                    # conv borders (written once; the quant stores only touch
                    # the interior): top pad row 0, bottom pad rows 57-59
                    # (57 = conv pad, 58/59 anti-NaN for the zero-pair
                    # overshoot), cols 0 and 57. Cols 58-63 only feed the
                    # unused alignment output columns.
                    nc.vector.memset(apad[i][:, 0, 0:58], 0)
                    nc.vector.memset(apad[i][:, 57:HP, :], 0)
                    nc.vector.memset(apad[i][:, 1:57, 0:1], 0)
                    nc.vector.memset(apad[i][:, 1:57, 57:58], 0)
                    for ci, (r0, rows) in enumerate(P1CH):
                        xt = xstg.tile([C, 14, W], F32, tag="xs", name="xs")
                        xv = xt[:, 0:rows, :]
                        nc.sync.dma_start(xv, x_in.ap()[i][:, r0:r0 + rows, :])
                        # u1 = round(15*x): RNE f32->i16, straight into cint
                        uv = cint[i][:, r0:r0 + rows, :]
                        nc.vector.tensor_scalar(uv, xv, 15.0, None, op0=ALU.mult)
                        # clip to [0,15] + exact int->fp8 convert; chunk 0
                        # on DVE (shortest path to group 0), rest on Pool
                        dsta = apad[i][:, 1 + r0:1 + r0 + rows, 1:W + 1]
                        eng = nc.vector if ci == 0 else nc.gpsimd
                        eng.tensor_scalar(dsta, uv, 15, 0, op0=ALU.min, op1=ALU.max)

                pe_keepalive(12)
                phase1_prep(0)
                nc.gpsimd.dma_start(tw1[:], w1_in.ap())
                nc.gpsimd.dma_start(tw2[:], w2_in.ap())
                phase1_prep(1)
                for i in range(nper):
                    if i + 2 < nper:
                        phase1_prep(i + 2)
                    conv(i, tw1, s1p)
                    if i >= 2:
                        sumsq(i - 2, ss1p, DSPLIT1)
                sumsq(nper - 2, ss1p, DSPLIT1)
                sumsq(nper - 1, ss1p, DSPLIT1)

            # ---------------- BN1 sync + coefs ----------------
            with nc.named_scope("bn1"):
                pe_keepalive(40)
                gst1 = stats_allreduce(s1p, ss1p, 1)
                uscale, ubias = _bn_coefs(nc, spool, gst1[:, 0:1], gst1[:, 1:2], cvcol, 1)

            # ---------------- phase2 (act_quant of BN1) + conv2 ----------------
            with nc.named_scope("conv2"):

                def phase2_prep(i):
                    # residual copy of x arrives as host-prepared bf16 while
                    # the DMA engines are otherwise idle. The load itself is
                    # dependency-free and would otherwise dispatch early and
                    # steal phase-1 DMA bandwidth (engines issue past blocked
                    # instructions), so gate it on the BN1 coefs via a dummy
                    # write into the tile.
                    nc.gpsimd.tensor_scalar(xbf[i][:, 0, 0:1], uscale[:], 0.0,
                                            None, op0=ALU.mult)
                    nc.sync.dma_start(xbf[i][:], xb_in.ap()[i])
                    for ci, (r0, rows) in enumerate(P2CH):
                        src = cint[i][:, r0:r0 + rows, :]
                        # u2 = round(uscale*y + ubias), RNE via i16 convert,
                        # in place over the conv1 ints (4x DVE)
                        nc.vector.tensor_scalar(src, src, uscale[:], ubias[:],
                                                op0=ALU.mult, op1=ALU.add)
                        dsta = apad[i][:, 1 + r0:1 + r0 + rows, 1:W + 1]
                        eng = nc.vector if ci == 0 else nc.gpsimd
                        eng.tensor_scalar(dsta, src, 15, 0, op0=ALU.min, op1=ALU.max)

                phase2_prep(0)
                phase2_prep(1)
                phase2_prep(2)
                phase2_prep(3)
                for i in range(nper):
                    if i + 4 < nper:
                        phase2_prep(i + 4)
                    conv(i, tw2, s2p)
                    if i >= 2:
                        sumsq(i - 2, ss2p, DSPLIT2)
                sumsq(nper - 2, ss2p, DSPLIT2)
                sumsq(nper - 1, ss2p, DSPLIT2)

            # ---------------- BN2 sync + coefs ----------------
            with nc.named_scope("bn2"):
                gst2 = stats_allreduce(s2p, ss2p, 2)
                fscale, fbias = _bn_coefs(nc, spool, gst2[:, 0:1], gst2[:, 1:2], cvcol, 2)

            # ---------------- finalize: BN2 + residual, streamed stores ----------------
            with nc.named_scope("finalize"):
                # per-image engine plan: two offload images run I on ACT and
                # J on Pool (14-row chunks); the rest run both on DVE with
                # 28-row chunks so the tail tracks the store-DMA rate
                PLAN = {2: (True, True), 5: (True, True)}
                for i in range(nper):
                    i_act, j_pool = PLAN.get(i, (False, False))
                    if i_act or j_pool:
                        chunks = [(r, 14) for r in range(0, H, 14)]
                    elif i == 0:
                        chunks = [(0, 14), (14, 14), (28, 28)]
                    else:
                        chunks = [(0, 28), (28, 28)]
                    for r0, rows in chunks:
                        src = cint[i][:, r0:r0 + rows, :]
                        if i_act or j_pool:
                            t2b = fopool.tile([C, 14, W], BF16, tag="fo", name="t2o")
                        else:
                            t2b = fpool.tile([C, 28, W], BF16, tag="fin", name="t2b")
                        t2 = t2b[:, 0:rows, :]
                        if i_act:
                            nc.scalar.activation(t2, src, ACTF.Identity,
                                                 bias=fbias[:], scale=fscale[:])
                        else:
                            nc.vector.tensor_scalar(t2, src, fscale[:], fbias[:],
                                                    op0=ALU.mult, op1=ALU.add)
                        # residual add in place; store completion recycles
                        # the tile
                        aeng = nc.gpsimd if j_pool else nc.vector
                        aeng.tensor_tensor(out=t2, in0=t2,
                                           in1=xbf[i][:, r0:r0 + rows, :], op=ALU.add)
                        nc.sync.dma_start(out_d.ap()[i][:, r0:r0 + rows, :], t2)

            if rep_ctx is not None:
                rep_ctx.__exit__(None, None, None)

    nc.compile()
    return nc


_PROG_CACHE = {}


def _get_program(ncores, nper):
    key = (ncores, nper)
    if key not in _PROG_CACHE:
        _PROG_CACHE[key] = build_program(ncores, nper)
    return _PROG_CACHE[key]


def make_inputs(x, w1, w2, gamma1, beta1, gamma2, beta2, ncores=NCORES):
    """Host-side prep: shard x (fp32 + bf16 residual copy), quantize weights,
    build cvec."""
    x = np.asarray(x, dtype=np.float32)
    n = x.shape[0]
    nper = n // ncores
    assert nper * ncores == n

    def wq(w):
        w = np.asarray(w, dtype=np.float32)
        alpha = np.float32(np.abs(w).max()) + np.float32(1e-12)
        q = np.round(np.clip(w / alpha, -1.0, 1.0) * np.float32(7.0))
        return q.astype(np.float32), np.float32(alpha)

    q1, a1 = wq(w1)
    q2, a2 = wq(w2)
    # [co, ci, kh, kw] -> [ci, 12, co]: 6 DoubleRow pairs
    # [(0,dw),(1,dw)] for dw=0..2 then [(2,dw), zero] for dw=0..2
    f8np = mybir.dt.np(F8)

    def pack(q):
        t = q.transpose(1, 2, 3, 0)     # [ci, kh, kw, co]
        z = np.zeros_like(t[:, 0, 0, :])
        rows = []
        for dw in range(3):
            rows += [t[:, 0, dw, :], t[:, 1, dw, :]]
        for dw in range(3):
            rows += [t[:, 2, dw, :], z]
        return np.ascontiguousarray(np.stack(rows, axis=1)).astype(f8np)

    w1s = pack(q1)
    w2s = pack(q2)
    s1 = np.float32(a1 / np.float32(105.0))
    s2 = np.float32(a2 / np.float32(105.0))
    m = np.float32(n * H * W)

    g1 = np.asarray(gamma1, dtype=np.float32)
    g2 = np.asarray(gamma2, dtype=np.float32)
    b1 = np.asarray(beta1, dtype=np.float32)
    b2 = np.asarray(beta2, dtype=np.float32)

    cvec = np.zeros((C, CV_NCOLS), dtype=np.float32)
    cvec[:, CV_INVM] = np.float32(1.0) / m
    cvec[:, CV_K1] = s1 * s1 / m
    cvec[:, CV_K2] = s2 * s2 / m
    cvec[:, CV_SM1] = s1 / m
    cvec[:, CV_SM2] = s2 / m
    cvec[:, CV_CA1] = np.float32(15.0) * s1 * g1
    cvec[:, CV_CB1] = np.float32(15.0) * b1
    cvec[:, CV_CA2] = s2 * g2
    cvec[:, CV_CB2] = b2
    cvec[:, CV_EPS] = 1e-5

    xbf = x.astype(ml_dtypes.bfloat16)
    in_maps = []
    for c in range(ncores):
        in_maps.append({
            "x": np.ascontiguousarray(x[c * nper:(c + 1) * nper]),
            "xbf": np.ascontiguousarray(xbf[c * nper:(c + 1) * nper]),
            "w1s": w1s, "w2s": w2s, "cvec": cvec,
        })
    return in_maps, nper


def run(x, w1, w2, gamma1, beta1, gamma2, beta2, trace=False):
    in_maps, nper = make_inputs(x, w1, w2, gamma1, beta1, gamma2, beta2)
    nc = _get_program(NCORES, nper)
    res = bass_utils.run_bass_kernel_spmd(
        nc, in_maps, core_ids=list(range(NCORES)), trace=trace)
    out = np.concatenate([r["out"] for r in res.results], axis=0).astype(np.float32)
    return out, res


def kernel(x, w1, w2, gamma1, beta1, gamma2, beta2):
    out, _ = run(x, w1, w2, gamma1, beta1, gamma2, beta2)
    return out


# revision 18
# speedup vs baseline: 1.1945x; 1.0126x over previous
"""Trainium2 Bass kernel for a quantized ResNet BasicBlock.

Reference computation (per reference.py):
    out = act_quant(x); out = conv3x3(out, weight_quant(w1)); out = BN(out, g1, b1)
    out = act_quant(out); out = conv3x3(out, weight_quant(w2)); out = BN(out, g2, b2)
    return out + x
with act_quant(x) = round(clip(x,0,1)*15)/15 (4-bit), weight_quant symmetric 4-bit
per-tensor (levels -7..7, scale alpha/7, alpha = max|w|), BN in training mode
(batch stats over (N,H,W)).

Strategy (8 NeuronCores, data-parallel over batch, sync-BN via AllReduce):
  * Quantized activations are integers 0..15, weights integers -7..7 - both
    exact in fp8e4m3, and fp32 PSUM accumulation never rounds, so each conv3x3
    is an EXACT integer computation.
  * Quantization rounds via the hardware's round-to-nearest float->int16
    convert (verified on hw): u1 = round(15*x) is one DVE tensor_scalar
    (f32 -> i16, written straight into the cint tile), u2 =
    round(uscale*y+ubias) one DVE tensor_scalar (i16 -> i16, per-channel
    fp32 scale/bias APs, 4x mode, in place over the conv-1 integers). The
    0..15 clamp+fp8 convert is one more tensor_scalar (i16 -> f8, min/max,
    exact for integers), mostly on GPSIMD.
  * conv3x3 over a zero-padded [C=128, 60, 64] fp8 image: per 8-row output
    group, 6 DoubleRow pair-matmuls contract all 9 taps as K=256 pairs:
    taps (0,dw)+(1,dw) pair naturally (pair stride = one 64B row), taps
    (2,dw) pair with an all-zero weight row (the paired garbage row
    multiplies 0). 42 DR passes per image-conv, no half-rate matmuls, no
    shifted-slab copies.
  * PSUM is split in three tiles (3+2+2 groups): PSUM WAR deps are
    tile-granular, so per-tile drains never gate another tile's matmuls and
    the next image's writes come a full rotation later. Drains run on ACT
    (Identity -> int16, exact) with the per-channel BN sum riding accum_out.
  * BN sum-of-squares is exact and split by rows: 3 DVE scalar_tensor_tensor
    pieces + one ACT Square piece (rows 40-47) that soaks up the ACT slack.
    (Row-subsampled variance was tried and rejected: adjacent-row correlation
    makes per-channel estimates ~3% noisy, an order over the error budget.)
    Per-channel stats are AllReduced across the 8 cores ([128,2] fp32), then
    BN+act_quant collapse into one per-channel scale/bias (Newton-iterated
    rsqrt on DVE, no ACT table reloads).
  * x is loaded as fp32 for quantization (quantizing a bf16 copy flips ~1% of
    the 4-bit levels and blows the error budget); a host-prepared bf16 copy
    of x is DMA-loaded during conv2 (DMA is idle there) as the residual.
  * Finalize: t2 = fscale*y2+fbias, out = t2 + x_bf16, streamed bf16 stores;
    per-image work is spread across DVE/ACT/Pool so the post-BN2 tail tracks
    the store-DMA rate. Dep-free DoubleRow matmuls into a junk PSUM bank
    bridge the BN1 sync bubble so the PE p-state governor never downclocks.
"""

import os
import sys

for _p in ("/opt/trn_rl_repo", "/root/.axon_site/_ro/trn_rl_repo"):
    if os.path.isdir(_p) and _p not in sys.path:
        sys.path.insert(0, _p)

import numpy as np
import ml_dtypes

import concourse.bass as bass  # noqa: F401  (registers types)
import concourse.tile as tile
from concourse import bacc, mybir
from concourse import bass_utils

F32 = mybir.dt.float32
BF16 = mybir.dt.bfloat16
I16 = mybir.dt.int16
F8 = mybir.dt.float8e4
ACTF = mybir.ActivationFunctionType
ALU = mybir.AluOpType
AX = mybir.AxisListType

C = 128
H = W = 56
HP = 60               # padded rows (57 = bottom conv pad, 58-59 anti-NaN for
                      # the zero-pair row overshoot)
WP = 64               # padded cols (16B-aligned rows for fp8 DoubleRow)
GR = 8                # output rows per PSUM group
NG = H // GR          # 7 groups per image
NCORES = 8

# cvec column indices (all [C] fp32, host-computed)
CV_INVM, CV_K1, CV_K2, CV_SM1, CV_SM2, CV_CA1, CV_CB1, CV_CA2, CV_CB2, \
    CV_EPS, CV_NCOLS = range(11)

# phase-1 row chunks (DMA + quant); group g's matmuls need image rows
# <= g*8+8, chunk 0 covering 14 rows releases group 0 immediately
P1CH = [(0, 14), (14, 14), (28, 14), (42, 14)]
# phase-2 row chunks: first chunk 12 rows so group-0 matmuls wait on one
# small chunk only
P2CH = [(0, 12), (12, 16), (28, 16), (44, 12)]


def _bn_coefs(nc, pool, S, SS, cvcol, ph):
    """[C,1] coef math from global integer-unit sum S / sumsq SS to the fused
    scale/bias for this BN + following op.

    ph=1: (uscale, ubias): u = round(conv_int*uscale + ubias) = 15*clip01(BN(y)).
    ph=2: (fscale, fbias): out = conv_int*fscale + fbias = BN(y2).
    """
    idx = [0]

    def mk():
        idx[0] += 1
        return pool.tile([C, 1], F32, tag=f"bc{ph}_{idx[0]}", name=f"bc{ph}_{idx[0]}")

    kcol = CV_K1 if ph == 1 else CV_K2
    smcol = CV_SM1 if ph == 1 else CV_SM2
    acol = CV_CA1 if ph == 1 else CV_CA2
    bcol = CV_CB1 if ph == 1 else CV_CB2

    # critical path: ms -> ms2 -> vpe -> seed -> newton x2 -> scale -> bias;
    # mean / sv / us0 hang off it in parallel (Pool).
    ms = mk()   # mean in real units, = S*(s/m)
    nc.vector.tensor_scalar(ms[:], S, cvcol(smcol), None, op0=ALU.mult)
    sv = mk()   # SS*s^2/m + eps  (on Pool, parallel)
    nc.gpsimd.tensor_scalar(sv[:], SS, cvcol(kcol), cvcol(CV_EPS),
                            op0=ALU.mult, op1=ALU.add)
    mean = mk()  # mean in integer units (only needed for the bias at the end)
    nc.gpsimd.tensor_scalar(mean[:], S, cvcol(CV_INVM), None, op0=ALU.mult)
    ms2 = mk()
    nc.vector.tensor_tensor(out=ms2[:], in0=ms[:], in1=ms[:], op=ALU.mult)
    vpe = mk()  # var_real + eps
    nc.vector.tensor_tensor(out=vpe[:], in0=sv[:], in1=ms2[:], op=ALU.subtract)
    # rsqrt without the scalar engine (avoids act-table reloads):
    # fast-inverse-sqrt seed j = MAGIC - (bits(v) >> 1) computed as
    # MAGIC - 0.5*bits(v) in the fp32 datapath, then two Newton steps
    # r <- r*(1.5 - 0.5*v*r^2) for ~5e-6 rel.
    seed = pool.tile([C, 1], mybir.dt.int32, tag=f"bseed{ph}", name=f"bseed{ph}")
    nc.vector.tensor_scalar(seed[:], vpe[:].bitcast(mybir.dt.int32), -0.5,
                            1597463007.0, op0=ALU.mult, op1=ALU.add)
    r0 = seed[:].bitcast(F32)
    t1 = mk()
    nc.vector.tensor_tensor(out=t1[:], in0=r0, in1=r0, op=ALU.mult)
    nc.vector.tensor_tensor(out=t1[:], in0=t1[:], in1=vpe[:], op=ALU.mult)
    nc.vector.tensor_scalar(t1[:], t1[:], -0.5, 1.5, op0=ALU.mult, op1=ALU.add)
    r1 = mk()
    nc.vector.tensor_tensor(out=r1[:], in0=r0, in1=t1[:], op=ALU.mult)
    # us0 = CA*r1 on Pool, parallel with the second Newton chain
    us0 = mk()
    nc.gpsimd.tensor_tensor(out=us0[:], in0=cvcol(acol), in1=r1[:], op=ALU.mult)
    t2 = mk()
    nc.vector.tensor_tensor(out=t2[:], in0=r1[:], in1=r1[:], op=ALU.mult)
    nc.vector.tensor_tensor(out=t2[:], in0=t2[:], in1=vpe[:], op=ALU.mult)
    nc.vector.tensor_scalar(t2[:], t2[:], -0.5, 1.5, op0=ALU.mult, op1=ALU.add)
    scale = mk()
    nc.vector.tensor_tensor(out=scale[:], in0=us0[:], in1=t2[:], op=ALU.mult)
    mb = mk()
    nc.vector.tensor_tensor(out=mb[:], in0=scale[:], in1=mean[:], op=ALU.mult)
    bias = mk()
    nc.vector.scalar_tensor_tensor(out=bias[:], in0=mb[:], scalar=-1.0,
                                   in1=cvcol(bcol), op0=ALU.mult, op1=ALU.add)
    return scale, bias


def build_program(ncores, nper, collective=True, reps=1):
    nc = bacc.Bacc("TRN2", target_bir_lowering=False, debug=False, num_devices=ncores)

    x_in = nc.dram_tensor("x", [nper, C, H, W], F32, kind="ExternalInput")
    xb_in = nc.dram_tensor("xbf", [nper, C, H, W], BF16, kind="ExternalInput")
    w1_in = nc.dram_tensor("w1s", [C, 12, C], F8, kind="ExternalInput")
    w2_in = nc.dram_tensor("w2s", [C, 12, C], F8, kind="ExternalInput")
    cv_in = nc.dram_tensor("cvec", [C, CV_NCOLS], F32, kind="ExternalInput")
    out_d = nc.dram_tensor("out", [nper, C, H, W], BF16, kind="ExternalOutput")

    with tile.TileContext(nc) as tc:
        with tc.tile_pool(name="const", bufs=1) as cpool, \
             tc.tile_pool(name="apad", bufs=nper) as apool, \
             tc.tile_pool(name="cint", bufs=nper) as ipool, \
             tc.tile_pool(name="xbf", bufs=nper) as xpool, \
             tc.tile_pool(name="xst", bufs=5) as xstg, \
             tc.tile_pool(name="sq", bufs=2) as sqpool, \
             tc.tile_pool(name="fin", bufs=4) as fpool, \
             tc.tile_pool(name="finoff", bufs=3) as fopool, \
             tc.tile_pool(name="stat", bufs=1) as spool, \
             tc.tile_pool(name="psum", bufs=1, space="PSUM") as ppool, \
             tc.tile_pool(name="dram", bufs=1, space="DRAM") as dpool:

            tw1 = cpool.tile([C, 12, C], F8, tag="w1")
            tw2 = cpool.tile([C, 12, C], F8, tag="w2")
            tcv = cpool.tile([C, CV_NCOLS], F32, tag="cv")
            # garbage-operand tile for PE p-state keep-alive matmuls (zeroed;
            # results land in junk PSUM rows and are never read)
            wdum = cpool.tile([C, 2, 512], F8, tag="wdum")
            # cvec via the ACT sequencer so SP's first x chunk issues in
            # parallel at t=0; weight DMAs via Pool's SWDGE
            nc.scalar.dma_start(tcv[:], cv_in.ap())

            def cvcol(j):
                return tcv[:, j:j + 1]

            # dep-free Identity op at t=0 soaks up the one-time act-table load
            warm = cpool.tile([C, 1], F32, tag="warm")
            nc.scalar.activation(warm[:], warm[:], ACTF.Identity, bias=0.0, scale=1.0)
            nc.vector.memset(wdum[:], 0)

            apad = [apool.tile([C, HP, WP], F8, tag="apad", name=f"apad{i}")
                    for i in range(nper)]
            cint = [ipool.tile([C, H, W], I16, tag="cint", name=f"cint{i}")
                    for i in range(nper)]
            xbf = [xpool.tile([C, H, W], BF16, tag="xbf", name=f"xbf{i}")
                   for i in range(nper)]

            rep_ctx = tc.For_i(0, reps, 1) if reps > 1 else None
            if rep_ctx is not None:
                rep_ctx.__enter__()

            # PSUM: three tiles (3+2+2 groups = 7 banks) + 1 junk bank
            GSPLIT = [(0, 3), (3, 2), (5, 2)]    # (first group, ngroups)
            pstiles = [ppool.tile([C, n * GR, WP], F32, tag=f"ps{j}",
                                  name=f"ps{j}", bufs=1)
                       for j, (_, n) in enumerate(GSPLIT)]
            # sumsq row pieces: (row0, rows, engine) per phase; phase 1's
            # last piece rides Pool (its B-chunks leave slack), phase 2's
            # stays on DVE (Pool carries more F there)
            DSPLIT1 = [(0, 24, 'v'), (24, 16, 'v'), (40, 8, 'a'), (48, 8, 'v')]
            DSPLIT2 = [(0, 24, 'v'), (24, 16, 'v'), (40, 8, 'a'), (48, 8, 'v')]
            ns_ = 3 * nper       # BN-sum columns (one per drain)
            nss = 4 * nper       # sumsq columns (one per D piece)
            s1p = spool.tile([C, ns_], F32, tag="s1p")
            ss1p = spool.tile([C, nss], F32, tag="ss1p")
            s2p = spool.tile([C, ns_], F32, tag="s1p", name="s2p")
            ss2p = spool.tile([C, nss], F32, tag="ss1p", name="ss2p")

            def pe_keepalive(n):
                out = pstiles[0][:, 0:GR, :].rearrange("c a b -> c (a b)")
                for k in range(n):
                    nc.tensor.matmul(out, wdum[:, :, 0:C], wdum[:],
                                     start=True, stop=True,
                                     perf_mode=mybir.MatmulPerfMode.DoubleRow)

            def emit_group(tw, flat, g, psview):
                """6 DoubleRow pair-matmuls for one 8-row output group:
                p=0..2 pair taps (0,p)+(1,p); p=3..5 pair tap (2,p-3) with an
                all-zero weight row."""
                out = psview.rearrange("c a b -> c (a b)")
                NFLAT = GR * WP
                for p in range(6):
                    dh = 0 if p < 3 else 2
                    dw = p if p < 3 else p - 3
                    base = (g * GR + dh) * WP + dw
                    rhs = flat[:, base:base + NFLAT]
                    rhs.ap.insert(1, [WP, 2])        # [C, 2, 512]
                    nc.tensor.matmul(out, tw[:, 2 * p:2 * p + 2, :], rhs,
                                     start=(p == 0), stop=(p == 5),
                                     perf_mode=mybir.MatmulPerfMode.DoubleRow)

            def conv(i, tw, sp):
                """conv3x3 of apad[i] into cint[i]; drains on ACT with the BN
                sum riding accum_out."""
                flat = apad[i].rearrange("c h w -> c (h w)")
                for j, (g0, ng) in enumerate(GSPLIT):
                    ps = pstiles[j]
                    for g in range(g0, g0 + ng):
                        emit_group(tw, flat, g, ps[:, (g - g0) * GR:(g - g0 + 1) * GR, :])
                    dst = cint[i][:, g0 * GR:(g0 + ng) * GR, :]
                    nc.scalar.activation(dst, ps[:, 0:ng * GR, 0:W], ACTF.Identity,
                                         bias=0.0, scale=1.0,
                                         accum_out=sp[:, 3 * i + j:3 * i + j + 1])

            def sumsq(i, ssp, dsplit):
                """Exact sum-of-squares pieces over cint[i]; emitted a couple
                of images late so the drain-gated pieces never head-of-line
                block the next images' quant ops in the DVE/Pool queues."""
                for dj, (r0, rows, eng) in enumerate(dsplit):
                    sub = cint[i][:, r0:r0 + rows, :]
                    kk = 4 * i + dj
                    if eng == 'a':
                        sq = sqpool.tile([C, 8, W], BF16, tag="sqa", name="sqa")
                        nc.scalar.activation(sq[:, 0:rows, :], sub, ACTF.Square,
                                             bias=0.0, scale=1.0,
                                             accum_out=ssp[:, kk:kk + 1])
                    else:
                        sq = sqpool.tile([C, 24, W], BF16, tag="sqd", name="sqd")
                        e = nc.vector if eng == 'v' else nc.gpsimd
                        e.scalar_tensor_tensor(
                            out=sq[:, 0:rows, :], in0=sub, scalar=1.0, in1=sub,
                            op0=ALU.mult, op1=ALU.mult,
                            accum_out=ssp[:, kk:kk + 1])

            def stats_allreduce(sp, ssp, tag):
                st = spool.tile([C, 2], F32, tag=f"st{tag}")
                nc.vector.tensor_reduce(out=st[:, 0:1], in_=sp[:], axis=AX.X, op=ALU.add)
                nc.vector.tensor_reduce(out=st[:, 1:2], in_=ssp[:], axis=AX.X, op=ALU.add)
                if not collective:
                    return st
                din = dpool.tile([C, 2], F32, tag=f"din{tag}")
                dout = dpool.tile([C, 2], F32, tag=f"dout{tag}")
                nc.gpsimd.dma_start(din[:], st[:])
                nc.gpsimd.collective_compute(
                    "AllReduce", ALU.add,
                    replica_groups=[list(range(ncores))],
                    ins=[din.opt()], outs=[dout.opt()])
                gst = spool.tile([C, 2], F32, tag=f"gst{tag}")
                nc.gpsimd.dma_start(gst[:], dout[:])
                return gst

            # ---------------- phase1 (act_quant of x) + conv1 ----------------
            with nc.named_scope("conv1"):

                def phase1_prep(i):
                    # conv borders (written once; the quant stores only touch
                    # the interior): top pad row 0, bottom pad rows 57-59
                    # (57 = conv pad, 58/59 anti-NaN for the zero-pair
                    # overshoot), cols 0 and 57. Cols 58-63 only feed the
                    # unused alignment output columns.
                    nc.vector.memset(apad[i][:, 0, 0:58], 0)
                    nc.vector.memset(apad[i][:, 57:HP, :], 0)
                    nc.vector.memset(apad[i][:, 1:57, 0:1], 0)
                    nc.vector.memset(apad[i][:, 1:57, 57:58], 0)
                    for ci, (r0, rows) in enumerate(P1CH):
                        xt = xstg.tile([C, 14, W], F32, tag="xs", name="xs")
                        xv = xt[:, 0:rows, :]
                        nc.sync.dma_start(xv, x_in.ap()[i][:, r0:r0 + rows, :])
                        # u1 = round(15*x): RNE f32->i16, straight into cint
                        uv = cint[i][:, r0:r0 + rows, :]
                        nc.vector.tensor_scalar(uv, xv, 15.0, None, op0=ALU.mult)
                        # clip to [0,15] + exact int->fp8 convert, on Pool
                        dsta = apad[i][:, 1 + r0:1 + r0 + rows, 1:W + 1]
                        nc.gpsimd.tensor_scalar(dsta, uv, 15, 0, op0=ALU.min, op1=ALU.max)

                pe_keepalive(12)
                phase1_prep(0)
                nc.gpsimd.dma_start(tw1[:], w1_in.ap())
                nc.gpsimd.dma_start(tw2[:], w2_in.ap())
                phase1_prep(1)
                for i in range(nper):
                    if i + 2 < nper:
                        phase1_prep(i + 2)
                    conv(i, tw1, s1p)
                    if i >= 2:
                        sumsq(i - 2, ss1p, DSPLIT1)
                sumsq(nper - 2, ss1p, DSPLIT1)
                sumsq(nper - 1, ss1p, DSPLIT1)

            # ---------------- BN1 sync + coefs ----------------
            with nc.named_scope("bn1"):
                pe_keepalive(30)
                gst1 = stats_allreduce(s1p, ss1p, 1)
                uscale, ubias = _bn_coefs(nc, spool, gst1[:, 0:1], gst1[:, 1:2], cvcol, 1)

            # ---------------- phase2 (act_quant of BN1) + conv2 ----------------
            with nc.named_scope("conv2"):

                def phase2_prep(i):
                    # residual copy of x arrives as host-prepared bf16 while
                    # the DMA engines are otherwise idle. The load itself is
                    # dependency-free and would otherwise dispatch early and
                    # steal phase-1 DMA bandwidth (engines issue past blocked
                    # instructions), so gate it on the BN1 coefs via a dummy
                    # write into the tile.
                    nc.gpsimd.tensor_scalar(xbf[i][:, 0, 0:1], uscale[:], 0.0,
                                            None, op0=ALU.mult)
                    nc.sync.dma_start(xbf[i][:], xb_in.ap()[i])
                    for ci, (r0, rows) in enumerate(P2CH):
                        src = cint[i][:, r0:r0 + rows, :]
                        # u2 = round(uscale*y + ubias), RNE via i16 convert,
                        # in place over the conv1 ints (4x DVE)
                        nc.vector.tensor_scalar(src, src, uscale[:], ubias[:],
                                                op0=ALU.mult, op1=ALU.add)
                        dsta = apad[i][:, 1 + r0:1 + r0 + rows, 1:W + 1]
                        eng = nc.vector if ci == 0 else nc.gpsimd
                        eng.tensor_scalar(dsta, src, 15, 0, op0=ALU.min, op1=ALU.max)

                phase2_prep(0)
                phase2_prep(1)
                phase2_prep(2)
                phase2_prep(3)
                for i in range(nper):
                    if i + 4 < nper:
                        phase2_prep(i + 4)
                    conv(i, tw2, s2p)
                    if i >= 2:
                        sumsq(i - 2, ss2p, DSPLIT2)
                sumsq(nper - 2, ss2p, DSPLIT2)
                sumsq(nper - 1, ss2p, DSPLIT2)

            # ---------------- BN2 sync + coefs ----------------
            with nc.named_scope("bn2"):
                gst2 = stats_allreduce(s2p, ss2p, 2)
                fscale, fbias = _bn_coefs(nc, spool, gst2[:, 0:1], gst2[:, 1:2], cvcol, 2)

            # ---------------- finalize: BN2 + residual, streamed stores ----------------
            with nc.named_scope("finalize"):
                # per-image engine plan: two offload images run I on ACT and
                # J on Pool (14-row chunks); the rest run both on DVE with
                # 28-row chunks so the tail tracks the store-DMA rate
                PLAN = {2: (True, True), 5: (True, True)}
                for i in range(nper):
                    i_act, j_pool = PLAN.get(i, (False, False))
                    if i_act or j_pool:
                        chunks = [(r, 14) for r in range(0, H, 14)]
                    elif i == 0:
                        chunks = [(0, 14), (14, 14), (28, 28)]
                    else:
                        chunks = [(0, 28), (28, 28)]
                    for r0, rows in chunks:
                        src = cint[i][:, r0:r0 + rows, :]
                        if i_act or j_pool:
                            t2b = fopool.tile([C, 14, W], BF16, tag="fo", name="t2o")
                        else:
                            t2b = fpool.tile([C, 28, W], BF16, tag="fin", name="t2b")
                        t2 = t2b[:, 0:rows, :]
                        if i_act:
                            nc.scalar.activation(t2, src, ACTF.Identity,
                                                 bias=fbias[:], scale=fscale[:])
                        else:
                            nc.vector.tensor_scalar(t2, src, fscale[:], fbias[:],
                                                    op0=ALU.mult, op1=ALU.add)
                        # residual add in place; store completion recycles
                        # the tile
                        aeng = nc.gpsimd if j_pool else nc.vector
                        aeng.tensor_tensor(out=t2, in0=t2,
                                           in1=xbf[i][:, r0:r0 + rows, :], op=ALU.add)
                        nc.sync.dma_start(out_d.ap()[i][:, r0:r0 + rows, :], t2)

            if rep_ctx is not None:
                rep_ctx.__exit__(None, None, None)

    nc.compile()
    return nc


_PROG_CACHE = {}


def _get_program(ncores, nper):
    key = (ncores, nper)
    if key not in _PROG_CACHE:
        _PROG_CACHE[key] = build_program(ncores, nper)
    return _PROG_CACHE[key]


def make_inputs(x, w1, w2, gamma1, beta1, gamma2, beta2, ncores=NCORES):
    """Host-side prep: shard x (fp32 + bf16 residual copy), quantize weights,
    build cvec."""
    x = np.asarray(x, dtype=np.float32)
    n = x.shape[0]
    nper = n // ncores
    assert nper * ncores == n

    def wq(w):
        w = np.asarray(w, dtype=np.float32)
        alpha = np.float32(np.abs(w).max()) + np.float32(1e-12)
        q = np.round(np.clip(w / alpha, -1.0, 1.0) * np.float32(7.0))
        return q.astype(np.float32), np.float32(alpha)

    q1, a1 = wq(w1)
    q2, a2 = wq(w2)
    # [co, ci, kh, kw] -> [ci, 12, co]: 6 DoubleRow pairs
    # [(0,dw),(1,dw)] for dw=0..2 then [(2,dw), zero] for dw=0..2
    f8np = mybir.dt.np(F8)

    def pack(q):
        t = q.transpose(1, 2, 3, 0)     # [ci, kh, kw, co]
        z = np.zeros_like(t[:, 0, 0, :])
        rows = []
        for dw in range(3):
            rows += [t[:, 0, dw, :], t[:, 1, dw, :]]
        for dw in range(3):
            rows += [t[:, 2, dw, :], z]
        return np.ascontiguousarray(np.stack(rows, axis=1)).astype(f8np)

    w1s = pack(q1)
    w2s = pack(q2)
    s1 = np.float32(a1 / np.float32(105.0))
    s2 = np.float32(a2 / np.float32(105.0))
    m = np.float32(n * H * W)

    g1 = np.asarray(gamma1, dtype=np.float32)
    g2 = np.asarray(gamma2, dtype=np.float32)
    b1 = np.asarray(beta1, dtype=np.float32)
    b2 = np.asarray(beta2, dtype=np.float32)

    cvec = np.zeros((C, CV_NCOLS), dtype=np.float32)
    cvec[:, CV_INVM] = np.float32(1.0) / m
    cvec[:, CV_K1] = s1 * s1 / m
    cvec[:, CV_K2] = s2 * s2 / m
    cvec[:, CV_SM1] = s1 / m
    cvec[:, CV_SM2] = s2 / m
    cvec[:, CV_CA1] = np.float32(15.0) * s1 * g1
    cvec[:, CV_CB1] = np.float32(15.0) * b1
    cvec[:, CV_CA2] = s2 * g2
    cvec[:, CV_CB2] = b2
    cvec[:, CV_EPS] = 1e-5

    xbf = x.astype(ml_dtypes.bfloat16)
    in_maps = []
    for c in range(ncores):
        in_maps.append({
            "x": np.ascontiguousarray(x[c * nper:(c + 1) * nper]),
            "xbf": np.ascontiguousarray(xbf[c * nper:(c + 1) * nper]),
            "w1s": w1s, "w2s": w2s, "cvec": cvec,
        })
    return in_maps, nper


def run(x, w1, w2, gamma1, beta1, gamma2, beta2, trace=False):
    in_maps, nper = make_inputs(x, w1, w2, gamma1, beta1, gamma2, beta2)
    nc = _get_program(NCORES, nper)
    res = bass_utils.run_bass_kernel_spmd(
        nc, in_maps, core_ids=list(range(NCORES)), trace=trace)
    out = np.concatenate([r["out"] for r in res.results], axis=0).astype(np.float32)
    return out, res


def kernel(x, w1, w2, gamma1, beta1, gamma2, beta2):
    out, _ = run(x, w1, w2, gamma1, beta1, gamma2, beta2)
    return out
